# revision 6
# baseline (speedup 1.0000x reference)
"""ChildSum TreeLSTM on 8 Trainium2 NeuronCores (Bass/Tile).

Data-parallel over the tree batch: each core owns B_LOCAL = 4 trees and runs a
level-synchronous scan over the shared full-binary-tree topology (leaves first,
root last).  The relabeled heap tree has children(i) = (2(i-256), 2(i-256)+1)
for internal node i, so every per-level child gather is a contiguous stride-2
range - no indirect addressing anywhere.

On-chip layout is feature-major: state tensors are [128 partitions, 2 feature
chunks, 2044 cols] with col = node*4 + tree.  Weights are pre-transposed on the
host (lhsT = W.T, 256-dim contraction on partitions) with gate blocks
reordered to [u, i, o] so sigmoid gates sit contiguously next to the f gates.
Inputs are pre-transposed/cast to fp16 on the host.  GEMMs run in fp16 with
fp32 PSUM accumulation; the c state stays fp32, h state and gates are fp16
(~1e-3 end-to-end rel err).

Big levels fuse the x-projection GEMMs into each level's PSUM accumulation
(x parts lead each accumulation group so they can hide behind the previous
level).  The serial tail (levels with <=32 nodes) instead uses x-projections
materialized once during the busy phase, which halves the tail's matmul count
and shortens each level's critical path to 2 DVE gate ops + 2 ACT ops.
"""

import numpy as np

B, N, K = 32, 511, 2
IN_DIM = MEM_DIM = 256
NCORES = 8
B_LOCAL = B // NCORES            # 4 trees per core
COLS = N * B_LOCAL               # 2044 columns, col = node*B_LOCAL + tree
LEAF_N = (N + 1) // 2            # 256 leaves
# (node_base, n_nodes) per level, leaves first
LEVELS = [(0, 256), (256, 128), (384, 64), (448, 32), (480, 16),
          (496, 8), (504, 4), (508, 2), (510, 1)]
CHUNK_NODES = 64                 # <=64 nodes (256 cols) per PSUM chunk
TAIL_BASE = 448                  # first tail node: levels with <=32 nodes
TAIL_N = N - TAIL_BASE           # 63 tail nodes, 252 cols

_PROGRAM_CACHE = {}


def _expected_children():
    ch = np.full((N, K), -1, dtype=np.int64)
    for p in range(N):
        for k in range(K):
            c = 2 * p + 1 + k
            if c < N:
                ch[N - 1 - p, k] = N - 1 - c
    return ch


def _np_fallback(inputs, W_ioux, b_ioux, W_iouh, b_iouh, W_fx, b_fx, W_fh, b_fh,
                 children_idx):
    """Generic (any-topology) numpy replica of the reference. Slow but exact."""
    sig = lambda x: 1.0 / (1.0 + np.exp(-x))
    x_iou = np.einsum('bni,oi->nbo', inputs, W_ioux) + b_ioux
    x_f = np.einsum('bni,mi->nbm', inputs, W_fx) + b_fx
    c_buf = np.zeros((N, B, MEM_DIM), np.float32)
    h_buf = np.zeros((N, B, MEM_DIM), np.float32)
    for i in range(N):
        ch = children_idx[i]
        valid = ch >= 0
        mask = valid.astype(np.float32)[:, None, None]
        safe = np.where(valid, ch, 0)
        child_c = c_buf[safe] * mask
        child_h = h_buf[safe] * mask
        h_sum = child_h.sum(0)
        iou = x_iou[i] + h_sum @ W_iouh.T + b_iouh
        ii, oo, uu = np.split(iou, 3, axis=-1)
        ii, oo, uu = sig(ii), sig(oo), np.tanh(uu)
        f = sig(np.einsum('kbm,nm->kbn', child_h, W_fh) + b_fh + x_f[i][None])
        c = ii * uu + (f * child_c).sum(0)
        h = oo * np.tanh(c)
        c_buf[i] = c
        h_buf[i] = h
    root_c = c_buf[N - 1]
    root_h = h_buf[N - 1]
    hiddens = np.ascontiguousarray(h_buf.transpose(1, 0, 2))
    return root_c, root_h, hiddens


def _build_program():
    import concourse.bacc as bacc
    import concourse.mybir as mybir
    import concourse.tile as tile

    F32 = mybir.dt.float32
    F16 = mybir.dt.float16
    AF = mybir.ActivationFunctionType

    nc = bacc.Bacc("TRN2", target_bir_lowering=False, debug=False)

    xT_d = nc.dram_tensor("xT", [2, 128, COLS], F16, kind="ExternalInput")
    wiouh_d = nc.dram_tensor("wiouh", [2, 128, 768], F16, kind="ExternalInput")
    wioux_d = nc.dram_tensor("wioux", [2, 128, 768], F16, kind="ExternalInput")
    wfh_d = nc.dram_tensor("wfh", [2, 128, 256], F16, kind="ExternalInput")
    wfx_d = nc.dram_tensor("wfx", [2, 128, 256], F16, kind="ExternalInput")
    biou_d = nc.dram_tensor("biou", [128, 6], F32, kind="ExternalInput")
    bf_d = nc.dram_tensor("bf", [128, 2], F32, kind="ExternalInput")
    hout_d = nc.dram_tensor("h_out", [128, 2, COLS], F16, kind="ExternalOutput")
    croot_d = nc.dram_tensor("c_root", [128, 2, B_LOCAL], F32, kind="ExternalOutput")

    with tile.TileContext(nc) as tc:
        with tc.tile_pool(name="state", bufs=1) as state, \
             tc.tile_pool(name="work", bufs=3) as work, \
             tc.tile_pool(name="iou_ps", bufs=2, space="PSUM") as iou_pool, \
             tc.tile_pool(name="f_ps", bufs=1, space="PSUM") as f_pool:

            xT = state.tile([128, 2, COLS], F16, tag="xT")
            wiouh = state.tile([128, 2, 768], F16, tag="wiouh")
            wioux = state.tile([128, 2, 768], F16, tag="wioux")
            wfh = state.tile([128, 2, 256], F16, tag="wfh")
            wfx = state.tile([128, 2, 256], F16, tag="wfx")
            biou = state.tile([128, 6], F32, tag="biou")
            bf = state.tile([128, 2], F32, tag="bf")
            h16 = state.tile([128, 2, COLS], F16, tag="h16")
            c32 = state.tile([128, 2, COLS], F32, tag="c32")
            TC = TAIL_N * B_LOCAL                           # 252 tail cols
            xb_iou = state.tile([128, 6, TC], F32, tag="xb_iou")
            xb_fb = state.tile([128, 2, 2 * TC], F32, tag="xb_fb")

            # load order: biases/weights needed first, then leaf cols chunk by
            # chunk so leaf compute starts early, then the rest.
            nc.sync.dma_start(biou[:], biou_d[:])
            nc.sync.dma_start(bf[:], bf_d[:])
            nc.sync.dma_start(wioux[:], wioux_d[:].rearrange("c p m -> p c m"))
            leaf_cols = LEAF_N * B_LOCAL
            for lo in range(0, leaf_cols, CHUNK_NODES * B_LOCAL):
                hi = lo + CHUNK_NODES * B_LOCAL
                nc.sync.dma_start(xT[:, :, lo:hi],
                                  xT_d[:, :, lo:hi].rearrange("c p n -> p c n"))
            nc.sync.dma_start(wiouh[:], wiouh_d[:].rearrange("c p m -> p c m"))
            nc.sync.dma_start(wfh[:], wfh_d[:].rearrange("c p m -> p c m"))
            nc.sync.dma_start(wfx[:], wfx_d[:].rearrange("c p m -> p c m"))
            nc.sync.dma_start(xT[:, :, leaf_cols:],
                              xT_d[:, :, leaf_cols:].rearrange("c p n -> p c n"))

            def gates_iou(ps, C):
                """PSUM [u,i,o] preactivations -> fp16 gates tile [128,6,C].

                io half first so ACT sigmoid overlaps the u-half DVE add."""
                gb = work.tile([128, 6, C], F32, tag="gb")
                bb = biou[:].unsqueeze(2).broadcast_to([128, 6, C])
                nc.vector.tensor_add(gb[:, 2:6], ps[:, 2:6], bb[:, 2:6])
                nc.vector.tensor_add(gb[:, 0:2], ps[:, 0:2], bb[:, 0:2])
                g = work.tile([128, 6, C], F16, tag="g")
                nc.scalar.activation(g[:, 2:6], gb[:, 2:6], AF.Sigmoid)
                nc.scalar.activation(g[:, 0:2], gb[:, 0:2], AF.Tanh)
                return g

            def c_h_tail(gi, go, gu, cols, C, fc_pair=None):
                """c = i*u (+ fc0 + fc1); h = o*tanh(c); writes c32/h16."""
                if fc_pair is None:
                    nc.vector.tensor_mul(c32[:, :, cols], gi, gu)
                else:
                    fc0, fc1 = fc_pair
                    iu = work.tile([128, 2, C], F16, tag="iu")
                    nc.vector.tensor_mul(iu[:], gi, gu)
                    tmp = work.tile([128, 2, C], F32, tag="ctmp")
                    nc.vector.tensor_add(tmp[:], fc0[:], fc1[:])
                    nc.vector.tensor_add(c32[:, :, cols], iu[:], tmp[:])
                tcb = work.tile([128, 2, C], F16, tag="tc")
                nc.scalar.activation(tcb[:], c32[:, :, cols], AF.Tanh)
                nc.vector.tensor_mul(h16[:, :, cols], go, tcb[:])

            def mm(ps_m, w, k, m, rhs, start, stop):
                nc.tensor.matmul(ps_m, w[:, k, m * 128:(m + 1) * 128], rhs,
                                 start=start, stop=stop)

            # ---------------- leaves (4 chunks of 64 nodes) ----------------
            for p0 in range(0, LEAF_N, CHUNK_NODES):
                C = CHUNK_NODES * B_LOCAL
                cols = slice(p0 * B_LOCAL, (p0 + CHUNK_NODES) * B_LOCAL)
                iou_ps = iou_pool.tile([128, 6, C], F32, tag="iou_ps")
                for m in range(6):
                    for k in range(2):
                        mm(iou_ps[:, m], wioux, k, m, xT[:, k, cols],
                           k == 0, k == 1)
                g = gates_iou(iou_ps, C)
                c_h_tail(g[:, 2:4], g[:, 4:6], g[:, 0:2], cols, C)
            nc.sync.dma_start(hout_d[:, :, 0:leaf_cols], h16[:, :, 0:leaf_cols])

            # ------- tail x-projections, materialized during busy phase -----
            tcols = slice(TAIL_BASE * B_LOCAL, N * B_LOCAL)
            xp = iou_pool.tile([128, 6, 256], F32, tag="iou_ps")
            for m in range(6):
                for k in range(2):
                    mm(xp[:, m, 0:TC], wioux, k, m, xT[:, k, tcols],
                       k == 0, k == 1)
            bb = biou[:].unsqueeze(2).broadcast_to([128, 6, TC])
            nc.vector.tensor_add(xb_iou[:], xp[:, :, 0:TC], bb)
            xpf = f_pool.tile([128, 2, 512], F32, tag="f_ps")
            for m in range(2):
                for k in range(2):
                    mm(xpf[:, m, 0:TC], wfx, k, m, xT[:, k, tcols],
                       k == 0, k == 1)
            # pre-broadcast the f x-part per child: [m, (node, kid, tree)]
            xbv = xb_fb[:].rearrange("p m (n k t) -> p m n k t", k=2, t=B_LOCAL)
            for m in range(2):
                nc.vector.tensor_scalar_add(
                    xbv[:, m], xpf[:, m, 0:TC]
                    .rearrange("p (n t) -> p n t", t=B_LOCAL)
                    .unsqueeze(2).broadcast_to([128, TAIL_N, 2, B_LOCAL]),
                    bf[:, m:m + 1])

            # ---------------- big internal levels (n > 32) -----------------
            for node_base, n_nodes in LEVELS[1:]:
                if node_base >= TAIL_BASE:
                    break
                for p0 in range(node_base, node_base + n_nodes, CHUNK_NODES):
                    cn = min(CHUNK_NODES, node_base + n_nodes - p0)
                    C = cn * B_LOCAL
                    cols = slice(p0 * B_LOCAL, (p0 + cn) * B_LOCAL)
                    q0 = 2 * (p0 - LEAF_N)
                    ch_cols = slice(q0 * B_LOCAL, (q0 + 2 * cn) * B_LOCAL)
                    hch = h16[:, :, ch_cols].rearrange(
                        "p c (n k t) -> p c n k t", k=2, t=B_LOCAL)
                    cch = c32[:, :, ch_cols].rearrange(
                        "p c (n k t) -> p c n k t", k=2, t=B_LOCAL)

                    hs = work.tile([128, 2, cn, B_LOCAL], F16, tag="hs")
                    nc.vector.tensor_add(hs[:], hch[:, :, :, 0], hch[:, :, :, 1])

                    # x parts first in each accumulation group: they only need
                    # xT, so the scheduler can run them during the previous
                    # level's elementwise phase.
                    iou_ps = iou_pool.tile([128, 6, C], F32, tag="iou_ps")
                    for m in range(6):
                        mm(iou_ps[:, m], wioux, 0, m, xT[:, 0, cols], True, False)
                        mm(iou_ps[:, m], wioux, 1, m, xT[:, 1, cols], False, False)
                        mm(iou_ps[:, m], wiouh, 0, m, hs[:, 0], False, False)
                        mm(iou_ps[:, m], wiouh, 1, m, hs[:, 1], False, True)

                    # f preactivations in child-col order [128, m, 2C]; x cols
                    # broadcast across the kid dim via a 0-step AP.
                    f_ps = f_pool.tile([128, 2, 2 * C], F32, tag="f_ps")
                    xbr = [xT[:, k, cols].rearrange("p (n t) -> p n t", t=B_LOCAL)
                           .unsqueeze(2).broadcast_to([128, cn, 2, B_LOCAL])
                           for k in range(2)]
                    for m in range(2):
                        mm(f_ps[:, m], wfx, 0, m, xbr[0], True, False)
                        mm(f_ps[:, m], wfx, 1, m, xbr[1], False, False)
                        mm(f_ps[:, m], wfh, 0, m, h16[:, 0, ch_cols], False, False)
                        mm(f_ps[:, m], wfh, 1, m, h16[:, 1, ch_cols], False, True)

                    fg = work.tile([128, 2, 2 * C], F16, tag="fg")
                    for m in range(2):
                        nc.scalar.activation(fg[:, m], f_ps[:, m], AF.Sigmoid,
                                             bias=bf[:, m:m + 1])

                    g = gates_iou(iou_ps, C)

                    fgv = fg[:].rearrange("p m (n k t) -> p m n k t",
                                          k=2, t=B_LOCAL)
                    fc0 = work.tile([128, 2, cn, B_LOCAL], F32, tag="fc0")
                    fc1 = work.tile([128, 2, cn, B_LOCAL], F32, tag="fc1")
                    nc.gpsimd.tensor_mul(fc0[:], fgv[:, :, :, 0], cch[:, :, :, 0])
                    nc.gpsimd.tensor_mul(fc1[:], fgv[:, :, :, 1], cch[:, :, :, 1])
                    c_h_tail(g[:, 2:4], g[:, 4:6], g[:, 0:2], cols, C,
                             fc_pair=(fc0, fc1))
                lv_cols = slice(node_base * B_LOCAL,
                                (node_base + n_nodes) * B_LOCAL)
                nc.sync.dma_start(hout_d[:, :, lv_cols], h16[:, :, lv_cols])

            # ------------------- tail levels (n <= 32) ---------------------
            # x parts come from xb_iou / xb_fb (biases included); each level
            # is 16 matmuls and a fused gate tile tg = [u(2C) | io(4C) | f(4C)]
            # built by 2 DVE adds, activated by 2 ACT ops.
            for node_base, n_nodes in LEVELS:
                if node_base < TAIL_BASE:
                    continue
                cn = n_nodes
                C = cn * B_LOCAL
                cols = slice(node_base * B_LOCAL, (node_base + cn) * B_LOCAL)
                r0 = (node_base - TAIL_BASE) * B_LOCAL
                q0 = 2 * (node_base - LEAF_N)
                ch_cols = slice(q0 * B_LOCAL, (q0 + 2 * cn) * B_LOCAL)
                hch = h16[:, :, ch_cols].rearrange(
                    "p c (n k t) -> p c n k t", k=2, t=B_LOCAL)
                cch = c32[:, :, ch_cols].rearrange(
                    "p c (n k t) -> p c n k t", k=2, t=B_LOCAL)

                hs = work.tile([128, 2, cn, B_LOCAL], F16, tag="hs")
                nc.vector.tensor_add(hs[:], hch[:, :, :, 0], hch[:, :, :, 1])

                iou_ps = iou_pool.tile([128, 6, C], F32, tag="iou_ps")
                for m in range(6):
                    mm(iou_ps[:, m], wiouh, 0, m, hs[:, 0], True, False)
                    mm(iou_ps[:, m], wiouh, 1, m, hs[:, 1], False, True)
                f_ps = f_pool.tile([128, 2, 2 * C], F32, tag="f_ps")
                for m in range(2):
                    mm(f_ps[:, m], wfh, 0, m, h16[:, 0, ch_cols], True, False)
                    mm(f_ps[:, m], wfh, 1, m, h16[:, 1, ch_cols], False, True)

                # tg = [u0 u1 | i0 i1 o0 o1 | f0(2C) f1(2C)]
                tg = work.tile([128, 10 * C], F32, tag="tg")
                nc.vector.tensor_add(
                    tg[0:128, 0:6 * C].rearrange("p (m n) -> p m n", m=6),
                    iou_ps[:], xb_iou[:, :, r0:r0 + C])
                nc.vector.tensor_add(
                    tg[0:128, 6 * C:10 * C].rearrange("p (m n) -> p m n", m=2),
                    f_ps[:], xb_fb[:, :, 2 * r0:2 * (r0 + C)])

                g16 = work.tile([128, 10 * C], F16, tag="g16")
                nc.scalar.activation(g16[0:128, 2 * C:], tg[0:128, 2 * C:],
                                     AF.Sigmoid)
                nc.scalar.activation(g16[0:128, 0:2 * C], tg[0:128, 0:2 * C],
                                     AF.Tanh)

                gu = g16[0:128, 0:2 * C].rearrange("p (c n) -> p c n", c=2)
                gi = g16[0:128, 2 * C:4 * C].rearrange("p (c n) -> p c n", c=2)
                go = g16[0:128, 4 * C:6 * C].rearrange("p (c n) -> p c n", c=2)
                fgv = g16[0:128, 6 * C:10 * C].rearrange(
                    "p (m n k t) -> p m n k t", m=2, k=2, t=B_LOCAL)
                fc0 = work.tile([128, 2, cn, B_LOCAL], F32, tag="fc0")
                fc1 = work.tile([128, 2, cn, B_LOCAL], F32, tag="fc1")
                nc.gpsimd.tensor_mul(fc0[:], fgv[:, :, :, 0], cch[:, :, :, 0])
                nc.gpsimd.tensor_mul(fc1[:], fgv[:, :, :, 1], cch[:, :, :, 1])
                c_h_tail(gi, go, gu, cols, C, fc_pair=(fc0, fc1))
                nc.sync.dma_start(hout_d[:, :, cols], h16[:, :, cols])

            nc.sync.dma_start(croot_d[:], c32[:, :, (N - 1) * B_LOCAL:])

    nc.compile()
    return nc


def _get_program():
    if "nc" not in _PROGRAM_CACHE:
        _PROGRAM_CACHE["nc"] = _build_program()
    return _PROGRAM_CACHE["nc"]


def _reorder_uio(w):
    """[768, ...] i,o,u row blocks -> u,i,o."""
    i, o, u = np.split(w, 3, axis=0)
    return np.concatenate([u, i, o], axis=0)


def _make_in_maps(inputs, W_ioux, b_ioux, W_iouh, b_iouh, W_fx, b_fx, W_fh,
                  b_fh):
    wiouh = np.ascontiguousarray(
        _reorder_uio(W_iouh).T.reshape(2, 128, 768)).astype(np.float16)
    wioux = np.ascontiguousarray(
        _reorder_uio(W_ioux).T.reshape(2, 128, 768)).astype(np.float16)
    wfh = np.ascontiguousarray(W_fh.T.reshape(2, 128, 256)).astype(np.float16)
    wfx = np.ascontiguousarray(W_fx.T.reshape(2, 128, 256)).astype(np.float16)
    biou = np.ascontiguousarray(
        _reorder_uio(b_ioux + b_iouh).reshape(6, 128).T)
    bfc = np.ascontiguousarray((b_fx + b_fh).reshape(2, 128).T)
    in_maps = []
    for c in range(NCORES):
        sl = inputs[c * B_LOCAL:(c + 1) * B_LOCAL]          # [4, N, 256]
        # -> [256, N, 4] -> [2, 128, COLS] with col = node*4 + tree
        xT = sl.transpose(2, 1, 0).reshape(2, 128, COLS).astype(np.float16)
        in_maps.append({
            "xT": np.ascontiguousarray(xT),
            "wiouh": wiouh, "wioux": wioux, "wfh": wfh, "wfx": wfx,
            "biou": biou, "bf": bfc,
        })
    return in_maps


def kernel(inputs, W_ioux, b_ioux, W_iouh, b_iouh, W_fx, b_fx, W_fh, b_fh,
           children_idx):
    from concourse.bass_utils import run_bass_kernel_spmd

    inputs = np.asarray(inputs, np.float32)
    W_ioux = np.asarray(W_ioux, np.float32)
    b_ioux = np.asarray(b_ioux, np.float32)
    W_iouh = np.asarray(W_iouh, np.float32)
    b_iouh = np.asarray(b_iouh, np.float32)
    W_fx = np.asarray(W_fx, np.float32)
    b_fx = np.asarray(b_fx, np.float32)
    W_fh = np.asarray(W_fh, np.float32)
    b_fh = np.asarray(b_fh, np.float32)
    children_idx = np.asarray(children_idx)

    if not np.array_equal(children_idx.astype(np.int64), _expected_children()):
        return _np_fallback(inputs, W_ioux, b_ioux, W_iouh, b_iouh, W_fx, b_fx,
                            W_fh, b_fh, children_idx)

    nc = _get_program()
    in_maps = _make_in_maps(inputs, W_ioux, b_ioux, W_iouh, b_iouh, W_fx, b_fx,
                            W_fh, b_fh)
    res = run_bass_kernel_spmd(nc, in_maps, list(range(NCORES)))

    root_c = np.empty((B, MEM_DIM), np.float32)
    hiddens = np.empty((B, N, MEM_DIM), np.float32)
    for c in range(NCORES):
        h = res.results[c]["h_out"].astype(np.float32).reshape(128, 2, N, B_LOCAL)
        hiddens[c * B_LOCAL:(c + 1) * B_LOCAL] = h.transpose(3, 2, 1, 0).reshape(
            B_LOCAL, N, MEM_DIM)
        cr = res.results[c]["c_root"]                        # [128, 2, 4]
        root_c[c * B_LOCAL:(c + 1) * B_LOCAL] = cr.transpose(2, 1, 0).reshape(
            B_LOCAL, MEM_DIM)
    root_h = np.ascontiguousarray(hiddens[:, N - 1])
    return root_c, root_h, hiddens
